# revision 10
# baseline (speedup 1.0000x reference)
"""Trainium2 Bass kernel for an AttentionBlock (GroupNorm + channel attention).

Computation (per batch element b, T = 32*32 spatial tokens, C = 512 channels,
H = 4 heads):
    h   = GroupNorm32(x)                      # stats over (T, C/32) per group
    qkv = h @ w_qkv + b_qkv                   # (T, H*C*3)
    per head: logits = q @ k^T / sqrt(T)      # over channel axis, (C, C)
              attn   = softmax(logits)
              av     = attn @ v               # (C, T)
    out = concat_heads(av)^T @ w_out + b_out + x

Sharding: data-parallel over batch, one batch element per NeuronCore (8 cores).
All matmuls run on the PE in float32r mode (full-rate fp32 at N>=256).

Layout strategy (per core):
  - x loaded natural (t on partitions), PE-transposed to xT (c on partitions)
  - GroupNorm stats per channel via bn_stats/bn_aggr on xT, group-aggregated
    across the 16 channels of a group with a block-diagonal ones matmul
    (result lands per-partition, already replicated within each group)
  - normalize in place -> hT; qkv GEMM contracts C on partitions:
      q, k computed token-major (lhsT = hT slices), v computed channel-major
      (lhsT = w_v slices) so no transposes are needed anywhere downstream
  - logits computed transposed (LT[d, c]) so softmax exp is elementwise and
    the attention value matmul can use ET slices directly as lhsT
  - softmax denominators via ones-matmul (per-partition), division folded into
    the PSUM->SBUF copy of av
  - output projection accumulated head by head into SBUF, residual added last
"""

import sys

if "/opt/trn_rl_repo" not in sys.path:
    sys.path.insert(0, "/opt/trn_rl_repo")

import numpy as np

import concourse.bass as bass  # noqa: F401  (kept for AP helpers)
import concourse.tile as tile
import concourse.mybir as mybir
from concourse import bacc
from concourse.bass_utils import run_bass_kernel_spmd
from concourse.masks import make_identity

B = 8
HSP = WSP = 32
T = HSP * WSP          # 1024
C = 512
H = 4
GROUPS = 32
GS = C // GROUPS       # 16
EPS = 1e-5
P = 128
CK = C // P            # 4 channel chunks (per head)
TK = T // P            # 8 token chunks
N_CORES = 8
F32 = mybir.dt.float32
F32R = mybir.dt.float32r

_PROGRAM_CACHE = {}


def _build_program(has_bq, has_bk, has_bv, has_bo):
    key = (has_bq, has_bk, has_bv, has_bo)
    if key in _PROGRAM_CACHE:
        return _PROGRAM_CACHE[key]

    nc = bacc.Bacc("TRN2", target_bir_lowering=False, debug=False,
                   num_devices=N_CORES)

    x_d = nc.dram_tensor("x", [T, C], F32, kind="ExternalInput").ap()
    wq_d = nc.dram_tensor("w_q", [C, H * C], F32R, kind="ExternalInput").ap()
    wk_d = nc.dram_tensor("w_k", [C, H * C], F32R, kind="ExternalInput").ap()
    wv_d = nc.dram_tensor("w_v", [C, H * C], F32R, kind="ExternalInput").ap()
    wo_d = nc.dram_tensor("w_o", [H * C, C], F32R, kind="ExternalInput").ap()
    gs_d = nc.dram_tensor("gs_col", [P, CK], F32, kind="ExternalInput").ap()
    gb_d = nc.dram_tensor("gb_col", [P, CK], F32, kind="ExternalInput").ap()
    m1_d = nc.dram_tensor("m1", [P, P], F32, kind="ExternalInput").ap()
    bv_d = (nc.dram_tensor("bv_col", [P, H * CK], F32, kind="ExternalInput").ap()
            if has_bv else None)
    bq_d = (nc.dram_tensor("b_q", [H * C], F32, kind="ExternalInput").ap()
            if has_bq else None)
    bk_d = (nc.dram_tensor("b_k", [H * C], F32, kind="ExternalInput").ap()
            if has_bk else None)
    bo_d = (nc.dram_tensor("b_o", [C], F32, kind="ExternalInput").ap()
            if has_bo else None)
    out_d = nc.dram_tensor("out", [T, C], F32, kind="ExternalOutput").ap()

    x_view = x_d.rearrange("(mt p) c -> p mt c", p=P)       # (128, 8, 512)
    out_view = out_d.rearrange("(mt p) c -> p mt c", p=P)   # (128, 8, 512)
    wq_view = wq_d.rearrange("(kk p) n -> p kk n", p=P)     # (128, 4, 2048)
    wk_view = wk_d.rearrange("(kk p) n -> p kk n", p=P)
    wv_view = wv_d.rearrange("(kk p) n -> p kk n", p=P)
    wo_view = wo_d.rearrange("(o p) n -> p o n", p=P)       # (128, 16, 512)

    from contextlib import ExitStack

    with tile.TileContext(nc) as tc, ExitStack() as ctx:
        ec = ctx.enter_context
        consts = ec(tc.tile_pool(name="consts", bufs=1))
        xp = ec(tc.tile_pool(name="xp", bufs=1))
        htp = ec(tc.tile_pool(name="htp", bufs=1))
        wqp = ec(tc.tile_pool(name="wqp", bufs=1))
        wkp = ec(tc.tile_pool(name="wkp", bufs=1))
        wvp = ec(tc.tile_pool(name="wvp", bufs=1))
        wop = ec(tc.tile_pool(name="wop", bufs=2))
        qp = ec(tc.tile_pool(name="qp", bufs=1))
        kp = ec(tc.tile_pool(name="kp", bufs=1))
        vp = ec(tc.tile_pool(name="vp", bufs=1))
        etp = ec(tc.tile_pool(name="etp", bufs=1))
        avp = ec(tc.tile_pool(name="avp", bufs=1))
        oap = ec(tc.tile_pool(name="oap", bufs=1))
        obp = ec(tc.tile_pool(name="obp", bufs=3))
        stat = ec(tc.tile_pool(name="stat", bufs=8))
        ps_mm = ec(tc.tile_pool(name="ps_mm", bufs=4, space="PSUM"))
        ps_t = ec(tc.tile_pool(name="ps_t", bufs=2, space="PSUM"))
        ps_sm = ec(tc.tile_pool(name="ps_sm", bufs=2, space="PSUM"))
        if True:
            # ---- constants -------------------------------------------------
            identity = consts.tile([P, P], F32)
            make_identity(nc, identity)
            # fp32r matmuls need even innermost free sizes -> use a 2-wide
            # ones matrix (memset doesn't support f32r; copy-cast from f32)
            ones_f32 = consts.tile([P, 2], F32)
            nc.vector.memset(ones_f32, 1.0)
            ones2 = consts.tile([P, 2], F32R)
            nc.vector.tensor_copy(ones2, ones_f32)
            eps_col = consts.tile([P, 1], F32)
            nc.vector.memset(eps_col, EPS)
            gs_col = consts.tile([P, CK], F32)
            nc.sync.dma_start(out=gs_col, in_=gs_d)
            gb_col = consts.tile([P, CK], F32)
            nc.sync.dma_start(out=gb_col, in_=gb_d)
            m1 = consts.tile([P, P], F32)
            nc.sync.dma_start(out=m1, in_=m1_d)
            if has_bv:
                bv_col = consts.tile([P, H * CK], F32)
                nc.sync.dma_start(out=bv_col, in_=bv_d)
            if has_bo:
                bo_bc = consts.tile([P, C], F32)
                nc.sync.dma_start(out=bo_bc, in_=bo_d.to_broadcast((P, C)))

            # ---- load x in halves; transpose + stats interleaved -----------
            x_sb = xp.tile([P, TK, C], F32)        # (128, 8, 512), t on part
            nc.sync.dma_start(out=x_sb[:, 0:4, :], in_=x_view[:, 0:4, :])
            nc.sync.dma_start(out=x_sb[:, 4:8, :], in_=x_view[:, 4:8, :])

            hT = htp.tile([P, CK, T], F32R)         # (128, 4, 1024), c on part
            sts = []
            for ck in range(CK):
                for mt in range(4):
                    pst = ps_t.tile([P, P], F32)
                    nc.tensor.transpose(
                        pst, x_sb[:, mt, ck * P:(ck + 1) * P], identity)
                    nc.any.tensor_copy(
                        out=hT[:, ck, mt * P:(mt + 1) * P], in_=pst)
                st = stat.tile([P, 2, 6], F32, tag=f"bn{ck}")
                sts.append(st)
                nc.vector.bn_stats(out=st[:, 0, :], in_=hT[:, ck, 0:512])
            for ck in range(CK):
                for mt in range(4, TK):
                    pst = ps_t.tile([P, P], F32)
                    nc.tensor.transpose(
                        pst, x_sb[:, mt, ck * P:(ck + 1) * P], identity)
                    nc.any.tensor_copy(
                        out=hT[:, ck, mt * P:(mt + 1) * P], in_=pst)
                st = sts[ck]
                nc.vector.bn_stats(out=st[:, 1, :], in_=hT[:, ck, 512:1024])
                mv = stat.tile([P, 2], F32, tag="mv")
                nc.vector.bn_aggr(out=mv, in_=st)
                # smat = [channel_mean, channel_E[x^2]]
                smat = stat.tile([P, 2], F32, tag="smat")
                nc.vector.tensor_copy(smat[:, 0:1], mv[:, 0:1])
                nc.vector.tensor_mul(smat[:, 1:2], mv[:, 0:1], mv[:, 0:1])
                nc.vector.tensor_add(smat[:, 1:2], smat[:, 1:2], mv[:, 1:2])
                # group-sum across the 16 channels of each group (replicated)
                gsp = ps_sm.tile([P, 2], F32, tag="small")
                nc.tensor.matmul(gsp, lhsT=m1, rhs=smat, start=True, stop=True)
                mg = stat.tile([P, 1], F32, tag="mg")
                nc.vector.tensor_scalar_mul(mg, gsp[:, 0:1], 1.0 / GS)
                msq = stat.tile([P, 1], F32, tag="msq")
                nc.vector.tensor_scalar_mul(msq, gsp[:, 1:2], 1.0 / GS)
                var = stat.tile([P, 1], F32, tag="var")
                nc.vector.tensor_mul(var, mg, mg)
                nc.vector.tensor_tensor(var, msq, var, mybir.AluOpType.subtract)
                # var <- rstd = 1/sqrt(var + eps)
                nc.scalar.activation(out=var, in_=var,
                                     func=mybir.ActivationFunctionType.Sqrt,
                                     bias=eps_col)
                nc.vector.reciprocal(var, var)
                am = stat.tile([P, 1], F32, tag="am")
                nc.vector.tensor_mul(am, var, gs_col[:, ck:ck + 1])
                bm = stat.tile([P, 1], F32, tag="bm")
                nc.vector.tensor_mul(bm, mg, am)
                nc.vector.tensor_tensor(bm, gb_col[:, ck:ck + 1], bm,
                                        mybir.AluOpType.subtract)
                nc.vector.tensor_scalar(out=hT[:, ck, :], in0=hT[:, ck, :],
                                        scalar1=am, scalar2=bm,
                                        op0=mybir.AluOpType.mult,
                                        op1=mybir.AluOpType.add)

            # ---- per-head attention pipeline -------------------------------
            import os as _os
            H_RUN = int(_os.environ.get("BASS_KERNEL_HEADS", str(H)))
            oacc = oap.tile([P, TK, C], F32)       # (128, 8, 512), t on part
            for h in range(H_RUN):
                wq_sb = wqp.tile([P, CK, C], F32R, tag="wq")
                nc.sync.dma_start(out=wq_sb, in_=wq_view[:, :, h * C:(h + 1) * C])
                wk_sb = wkp.tile([P, CK, C], F32R, tag="wk")
                nc.sync.dma_start(out=wk_sb, in_=wk_view[:, :, h * C:(h + 1) * C])
                wv_sb = wvp.tile([P, CK, C], F32R, tag="wv")
                nc.sync.dma_start(out=wv_sb, in_=wv_view[:, :, h * C:(h + 1) * C])
                wo_sb = wop.tile([P, CK, C], F32R, tag="wo")
                nc.sync.dma_start(out=wo_sb, in_=wo_view[:, h * CK:(h + 1) * CK, :])
                if has_bq:
                    bq_bc = stat.tile([P, C], F32, tag="bqbc")
                    nc.sync.dma_start(
                        out=bq_bc, in_=bq_d[h * C:(h + 1) * C].to_broadcast((P, C)))
                if has_bk:
                    bk_bc = stat.tile([P, C], F32, tag="bkbc")
                    nc.sync.dma_start(
                        out=bk_bc, in_=bk_d[h * C:(h + 1) * C].to_broadcast((P, C)))

                # q, k token-major: (128, 8, 512)
                q_sb = qp.tile([P, TK, C], F32R, tag="q")
                k_sb = kp.tile([P, TK, C], F32R, tag="k")
                for dst, w_sb, bias_bc in (
                    (q_sb, wq_sb, "bq"), (k_sb, wk_sb, "bk"),
                ):
                    for mt in range(TK):
                        ps = ps_mm.tile([P, 512], F32, tag="mm")
                        for kk in range(CK):
                            nc.tensor.matmul(
                                ps,
                                lhsT=(hT[:, kk, mt * P:(mt + 1) * P]),
                                rhs=(w_sb[:, kk, :]),
                                start=(kk == 0), stop=(kk == CK - 1))
                        if bias_bc == "bq" and has_bq:
                            nc.vector.tensor_add(dst[:, mt, :], ps, bq_bc)
                        elif bias_bc == "bk" and has_bk:
                            nc.vector.tensor_add(dst[:, mt, :], ps, bk_bc)
                        else:
                            nc.any.tensor_copy(out=dst[:, mt, :], in_=ps)

                # v channel-major: (128, 4, 1024)
                vT_sb = vp.tile([P, CK, T], F32R, tag="v")
                for ck in range(CK):
                    for tw in range(2):
                        ps = ps_mm.tile([P, 512], F32, tag="mm")
                        for kk in range(CK):
                            nc.tensor.matmul(
                                ps,
                                lhsT=(wv_sb[:, kk, ck * P:(ck + 1) * P]),
                                rhs=(hT[:, kk, tw * 512:(tw + 1) * 512]),
                                start=(kk == 0), stop=(kk == CK - 1))
                        dst = vT_sb[:, ck, tw * 512:(tw + 1) * 512]
                        if has_bv:
                            nc.vector.tensor_scalar_add(
                                dst, ps, bv_col[:, h * CK + ck:h * CK + ck + 1])
                        else:
                            nc.any.tensor_copy(out=dst, in_=ps)

                # logits^T (d on partitions), exp fused with 1/sqrt(T) scale
                et_sb = etp.tile([P, CK, C], F32R, tag="et")
                for dk in range(CK):
                    ps = ps_mm.tile([P, 512], F32, tag="mm")
                    for mt in range(TK):
                        nc.tensor.matmul(
                            ps,
                            lhsT=(k_sb[:, mt, dk * P:(dk + 1) * P]),
                            rhs=(q_sb[:, mt, :]),
                            start=(mt == 0), stop=(mt == TK - 1))
                    nc.scalar.activation(
                        out=et_sb[:, dk, :], in_=ps,
                        func=mybir.ActivationFunctionType.Exp,
                        scale=1.0 / 32.0)

                # softmax denominators, reciprocal per partition (c)
                rs = stat.tile([P, CK], F32, tag="rs")
                for ck in range(CK):
                    ps1 = ps_sm.tile([P, 2], F32, tag="small")
                    for dk in range(CK):
                        nc.tensor.matmul(
                            ps1,
                            lhsT=(et_sb[:, dk, ck * P:(ck + 1) * P]),
                            rhs=(ones2),
                            start=(dk == 0), stop=(dk == CK - 1))
                    nc.vector.reciprocal(rs[:, ck:ck + 1], ps1[:, 0:1])

                # av = (E @ v) / s, channel-major (128, 4, 1024)
                av_sb = avp.tile([P, CK, T], F32R, tag="av")
                for ck in range(CK):
                    for tw in range(2):
                        ps = ps_mm.tile([P, 512], F32, tag="mm")
                        for dk in range(CK):
                            nc.tensor.matmul(
                                ps,
                                lhsT=(et_sb[:, dk, ck * P:(ck + 1) * P]),
                                rhs=(vT_sb[:, dk, tw * 512:(tw + 1) * 512]),
                                start=(dk == 0), stop=(dk == CK - 1))
                        nc.vector.tensor_scalar_mul(
                            av_sb[:, ck, tw * 512:(tw + 1) * 512], ps,
                            rs[:, ck:ck + 1])

                # partial output projection for this head
                for mt in range(TK):
                    ps = ps_mm.tile([P, 512], F32, tag="mm")
                    for ck in range(CK):
                        nc.tensor.matmul(
                            ps,
                            lhsT=(av_sb[:, ck, mt * P:(mt + 1) * P]),
                            rhs=(wo_sb[:, ck, :]),
                            start=(ck == 0), stop=(ck == CK - 1))
                    if h == 0:
                        nc.any.tensor_copy(out=oacc[:, mt, :], in_=ps)
                    elif h < H_RUN - 1:
                        nc.vector.tensor_add(oacc[:, mt, :], oacc[:, mt, :], ps)
                    else:
                        # last head: fold in residual and store immediately
                        ob = obp.tile([P, C], F32, tag="ob")
                        nc.vector.tensor_add(ob, oacc[:, mt, :], ps)
                        nc.vector.tensor_add(ob, ob, x_sb[:, mt, :])
                        if has_bo:
                            nc.vector.tensor_add(ob, ob, bo_bc)
                        nc.sync.dma_start(out=out_view[:, mt, :], in_=ob)

    nc.compile()
    _PROGRAM_CACHE[key] = nc
    return nc


def kernel(x, gn_scale, gn_bias, w_qkv, b_qkv, w_out, b_out, **_unused):
    x = np.ascontiguousarray(np.asarray(x, np.float32)).reshape(B, T, C)
    gn_scale = np.asarray(gn_scale, np.float32)
    gn_bias = np.asarray(gn_bias, np.float32)
    w_qkv = np.asarray(w_qkv, np.float32)
    b_qkv = np.asarray(b_qkv, np.float32)
    w_out = np.ascontiguousarray(np.asarray(w_out, np.float32))
    b_out = np.asarray(b_out, np.float32)

    # de-interleave qkv weights: column j of w_qkv maps to
    # (head = j // (C*3), channel = (j // 3) % C, which = j % 3)
    w4 = w_qkv.reshape(C, H, C, 3)
    w_q = np.ascontiguousarray(w4[..., 0].reshape(C, H * C))
    w_k = np.ascontiguousarray(w4[..., 1].reshape(C, H * C))
    w_v = np.ascontiguousarray(w4[..., 2].reshape(C, H * C))
    b4 = b_qkv.reshape(H, C, 3)
    b_q = np.ascontiguousarray(b4[..., 0].reshape(H * C))
    b_k = np.ascontiguousarray(b4[..., 1].reshape(H * C))
    b_v = np.ascontiguousarray(b4[..., 2].reshape(H * C))

    has_bq = bool(np.any(b_q))
    has_bk = bool(np.any(b_k))
    has_bv = bool(np.any(b_v))
    has_bo = bool(np.any(b_out))

    gs_col = np.ascontiguousarray(gn_scale.reshape(CK, P).T)
    gb_col = np.ascontiguousarray(gn_bias.reshape(CK, P).T)
    m1 = np.kron(np.eye(P // GS, dtype=np.float32),
                 np.ones((GS, GS), np.float32))

    nc = _build_program(has_bq, has_bk, has_bv, has_bo)

    shared = {
        "w_q": w_q, "w_k": w_k, "w_v": w_v, "w_o": w_out,
        "gs_col": gs_col, "gb_col": gb_col, "m1": m1,
    }
    if has_bv:
        shared["bv_col"] = np.ascontiguousarray(b_v.reshape(H * CK, P).T)
    if has_bq:
        shared["b_q"] = b_q
    if has_bk:
        shared["b_k"] = b_k
    if has_bo:
        shared["b_o"] = b_out

    in_maps = [dict(shared, x=np.ascontiguousarray(x[i]))
               for i in range(N_CORES)]

    import os
    trace = bool(int(os.environ.get("BASS_KERNEL_TRACE", "0")))
    kwargs = {}
    if trace:
        tmpdir = os.environ.get("BASS_KERNEL_TRACE_DIR")
        if tmpdir:
            os.makedirs(tmpdir, exist_ok=True)
            kwargs["tmpdir"] = tmpdir
    res = run_bass_kernel_spmd(nc, in_maps, list(range(N_CORES)),
                               trace=trace, **kwargs)
    if trace:
        global LAST_EXEC_TIME_NS
        LAST_EXEC_TIME_NS = res.exec_time_ns
        print(f"HW exec time: {res.exec_time_ns} ns")

    out = np.stack([res.results[i]["out"] for i in range(N_CORES)], axis=0)
    return out.reshape(B, HSP, WSP, C)


LAST_EXEC_TIME_NS = None


# revision 11
# speedup vs baseline: 1.1854x; 1.1854x over previous
"""Trainium2 Bass kernel for an AttentionBlock (GroupNorm + channel attention).

Computation (per batch element b, T = 32*32 spatial tokens, C = 512 channels,
H = 4 heads):
    h   = GroupNorm32(x)                      # stats over (T, C/32) per group
    qkv = h @ w_qkv + b_qkv                   # (T, H*C*3)
    per head: logits = q @ k^T / sqrt(T)      # over channel axis, (C, C)
              attn   = softmax(logits)
              av     = attn @ v               # (C, T)
    out = concat_heads(av)^T @ w_out + b_out + x

Sharding: data-parallel over batch, one batch element per NeuronCore (8 cores).
All matmuls run on the PE in float32r mode (full-rate fp32 at N>=256).

Layout strategy (per core):
  - x loaded natural (t on partitions), PE-transposed to xT (c on partitions)
  - GroupNorm stats per channel via bn_stats/bn_aggr on xT, group-aggregated
    across the 16 channels of a group with a block-diagonal ones matmul
    (result lands per-partition, already replicated within each group)
  - normalize in place -> hT; qkv GEMM contracts C on partitions:
      q, k computed token-major (lhsT = hT slices), v computed channel-major
      (lhsT = w_v slices) so no transposes are needed anywhere downstream
  - logits computed transposed (LT[d, c]) so softmax exp is elementwise and
    the attention value matmul can use ET slices directly as lhsT
  - softmax denominators via ones-matmul (per-partition), division folded into
    the PSUM->SBUF copy of av
  - output projection accumulated head by head into SBUF, residual added last
"""

import sys

if "/opt/trn_rl_repo" not in sys.path:
    sys.path.insert(0, "/opt/trn_rl_repo")

import numpy as np

import concourse.bass as bass  # noqa: F401  (kept for AP helpers)
import concourse.tile as tile
import concourse.mybir as mybir
from concourse import bacc
from concourse.bass_utils import run_bass_kernel_spmd
from concourse.masks import make_identity

B = 8
HSP = WSP = 32
T = HSP * WSP          # 1024
C = 512
H = 4
GROUPS = 32
GS = C // GROUPS       # 16
EPS = 1e-5
P = 128
CK = C // P            # 4 channel chunks (per head)
TK = T // P            # 8 token chunks
N_CORES = 8
F32 = mybir.dt.float32
F32R = mybir.dt.float32r

_PROGRAM_CACHE = {}


def _build_program(has_bq, has_bk, has_bv, has_bo):
    key = (has_bq, has_bk, has_bv, has_bo)
    if key in _PROGRAM_CACHE:
        return _PROGRAM_CACHE[key]

    nc = bacc.Bacc("TRN2", target_bir_lowering=False, debug=False,
                   num_devices=N_CORES)

    x_d = nc.dram_tensor("x", [T, C], F32, kind="ExternalInput").ap()
    wq_d = nc.dram_tensor("w_q", [C, H * C], F32R, kind="ExternalInput").ap()
    wk_d = nc.dram_tensor("w_k", [C, H * C], F32R, kind="ExternalInput").ap()
    wv_d = nc.dram_tensor("w_v", [C, H * C], F32R, kind="ExternalInput").ap()
    wo_d = nc.dram_tensor("w_o", [H * C, C], F32R, kind="ExternalInput").ap()
    gs_d = nc.dram_tensor("gs_col", [P, CK], F32, kind="ExternalInput").ap()
    gb_d = nc.dram_tensor("gb_col", [P, CK], F32, kind="ExternalInput").ap()
    m1_d = nc.dram_tensor("m1", [P, P], F32, kind="ExternalInput").ap()
    bv_d = (nc.dram_tensor("bv_col", [P, H * CK], F32, kind="ExternalInput").ap()
            if has_bv else None)
    bq_d = (nc.dram_tensor("b_q", [H * C], F32, kind="ExternalInput").ap()
            if has_bq else None)
    bk_d = (nc.dram_tensor("b_k", [H * C], F32, kind="ExternalInput").ap()
            if has_bk else None)
    bo_d = (nc.dram_tensor("b_o", [C], F32, kind="ExternalInput").ap()
            if has_bo else None)
    out_d = nc.dram_tensor("out", [T, C], F32, kind="ExternalOutput").ap()

    x_view = x_d.rearrange("(mt p) c -> p mt c", p=P)       # (128, 8, 512)
    out_view = out_d.rearrange("(mt p) c -> p mt c", p=P)   # (128, 8, 512)
    wq_view = wq_d.rearrange("(kk p) n -> p kk n", p=P)     # (128, 4, 2048)
    wk_view = wk_d.rearrange("(kk p) n -> p kk n", p=P)
    wv_view = wv_d.rearrange("(kk p) n -> p kk n", p=P)
    wo_view = wo_d.rearrange("(o p) n -> p o n", p=P)       # (128, 16, 512)

    from contextlib import ExitStack

    with tile.TileContext(nc) as tc, ExitStack() as ctx:
        ec = ctx.enter_context
        consts = ec(tc.tile_pool(name="consts", bufs=1))
        xp = ec(tc.tile_pool(name="xp", bufs=1))
        htp = ec(tc.tile_pool(name="htp", bufs=1))
        wqp = ec(tc.tile_pool(name="wqp", bufs=1))
        wkp = ec(tc.tile_pool(name="wkp", bufs=1))
        wvp = ec(tc.tile_pool(name="wvp", bufs=1))
        wop = ec(tc.tile_pool(name="wop", bufs=2))
        qp = ec(tc.tile_pool(name="qp", bufs=1))
        kp = ec(tc.tile_pool(name="kp", bufs=1))
        vp = ec(tc.tile_pool(name="vp", bufs=1))
        etp = ec(tc.tile_pool(name="etp", bufs=1))
        avp = ec(tc.tile_pool(name="avp", bufs=1))
        oap = ec(tc.tile_pool(name="oap", bufs=1))
        obp = ec(tc.tile_pool(name="obp", bufs=3))
        stat = ec(tc.tile_pool(name="stat", bufs=8))
        ps_mm = ec(tc.tile_pool(name="ps_mm", bufs=4, space="PSUM"))
        ps_t = ec(tc.tile_pool(name="ps_t", bufs=2, space="PSUM"))
        ps_sm = ec(tc.tile_pool(name="ps_sm", bufs=2, space="PSUM"))
        if True:
            # ---- constants -------------------------------------------------
            identity = consts.tile([P, P], F32)
            make_identity(nc, identity)
            # fp32r matmuls need even innermost free sizes -> use a 2-wide
            # ones matrix (memset doesn't support f32r; copy-cast from f32)
            ones_f32 = consts.tile([P, 2], F32)
            nc.vector.memset(ones_f32, 1.0)
            ones2 = consts.tile([P, 2], F32R)
            nc.vector.tensor_copy(ones2, ones_f32)
            eps_col = consts.tile([P, 1], F32)
            nc.vector.memset(eps_col, EPS)
            gs_col = consts.tile([P, CK], F32)
            nc.sync.dma_start(out=gs_col, in_=gs_d)
            gb_col = consts.tile([P, CK], F32)
            nc.sync.dma_start(out=gb_col, in_=gb_d)
            m1 = consts.tile([P, P], F32)
            nc.sync.dma_start(out=m1, in_=m1_d)
            if has_bv:
                bv_col = consts.tile([P, H * CK], F32)
                nc.sync.dma_start(out=bv_col, in_=bv_d)
            if has_bo:
                bo_bc = consts.tile([P, C], F32)
                nc.sync.dma_start(out=bo_bc, in_=bo_d.to_broadcast((P, C)))

            # ---- load x (natural layout) and transpose to xT ---------------
            x_sb = xp.tile([P, TK, C], F32)        # (128, 8, 512), t on part
            nc.sync.dma_start(out=x_sb[:, 0:4, :], in_=x_view[:, 0:4, :])
            nc.sync.dma_start(out=x_sb[:, 4:8, :], in_=x_view[:, 4:8, :])

            hT = htp.tile([P, CK, T], F32R)         # (128, 4, 1024), c on part
            for mt in range(TK):
                for ck in range(CK):
                    pst = ps_t.tile([P, P], F32)
                    nc.tensor.transpose(
                        pst, x_sb[:, mt, ck * P:(ck + 1) * P], identity)
                    nc.any.tensor_copy(
                        out=hT[:, ck, mt * P:(mt + 1) * P], in_=pst)

            # ---- GroupNorm stats + normalize (hT in place) -----------------
            for ck in range(CK):
                st = stat.tile([P, 2, 6], F32, tag="bn")
                nc.vector.bn_stats(out=st[:, 0, :], in_=hT[:, ck, 0:512])
                nc.vector.bn_stats(out=st[:, 1, :], in_=hT[:, ck, 512:1024])
                mv = stat.tile([P, 2], F32, tag="mv")
                nc.vector.bn_aggr(out=mv, in_=st)
                # smat = [channel_mean, channel_E[x^2]]
                smat = stat.tile([P, 2], F32, tag="smat")
                nc.vector.tensor_copy(smat[:, 0:1], mv[:, 0:1])
                nc.vector.tensor_mul(smat[:, 1:2], mv[:, 0:1], mv[:, 0:1])
                nc.vector.tensor_add(smat[:, 1:2], smat[:, 1:2], mv[:, 1:2])
                # group-sum across the 16 channels of each group (replicated)
                gsp = ps_sm.tile([P, 2], F32, tag="small")
                nc.tensor.matmul(gsp, lhsT=m1, rhs=smat, start=True, stop=True)
                mg = stat.tile([P, 1], F32, tag="mg")
                nc.vector.tensor_scalar_mul(mg, gsp[:, 0:1], 1.0 / GS)
                msq = stat.tile([P, 1], F32, tag="msq")
                nc.vector.tensor_scalar_mul(msq, gsp[:, 1:2], 1.0 / GS)
                var = stat.tile([P, 1], F32, tag="var")
                nc.vector.tensor_mul(var, mg, mg)
                nc.vector.tensor_tensor(var, msq, var, mybir.AluOpType.subtract)
                # var <- rstd = 1/sqrt(var + eps)
                nc.scalar.activation(out=var, in_=var,
                                     func=mybir.ActivationFunctionType.Sqrt,
                                     bias=eps_col)
                nc.vector.reciprocal(var, var)
                am = stat.tile([P, 1], F32, tag="am")
                nc.vector.tensor_mul(am, var, gs_col[:, ck:ck + 1])
                bm = stat.tile([P, 1], F32, tag="bm")
                nc.vector.tensor_mul(bm, mg, am)
                nc.vector.tensor_tensor(bm, gb_col[:, ck:ck + 1], bm,
                                        mybir.AluOpType.subtract)
                nc.vector.tensor_scalar(out=hT[:, ck, :], in0=hT[:, ck, :],
                                        scalar1=am, scalar2=bm,
                                        op0=mybir.AluOpType.mult,
                                        op1=mybir.AluOpType.add)

            # ---- per-head attention pipeline -------------------------------
            import os as _os
            H_RUN = int(_os.environ.get("BASS_KERNEL_HEADS", str(H)))
            oacc = oap.tile([P, TK, C], F32)       # (128, 8, 512), t on part
            for h in range(H_RUN):
                wq_sb = wqp.tile([P, CK, C], F32R, tag="wq")
                nc.sync.dma_start(out=wq_sb, in_=wq_view[:, :, h * C:(h + 1) * C])
                wk_sb = wkp.tile([P, CK, C], F32R, tag="wk")
                nc.sync.dma_start(out=wk_sb, in_=wk_view[:, :, h * C:(h + 1) * C])
                wv_sb = wvp.tile([P, CK, C], F32R, tag="wv")
                nc.sync.dma_start(out=wv_sb, in_=wv_view[:, :, h * C:(h + 1) * C])
                wo_sb = wop.tile([P, CK, C], F32R, tag="wo")
                nc.sync.dma_start(out=wo_sb, in_=wo_view[:, h * CK:(h + 1) * CK, :])
                if has_bq:
                    bq_bc = stat.tile([P, C], F32, tag="bqbc")
                    nc.sync.dma_start(
                        out=bq_bc, in_=bq_d[h * C:(h + 1) * C].to_broadcast((P, C)))
                if has_bk:
                    bk_bc = stat.tile([P, C], F32, tag="bkbc")
                    nc.sync.dma_start(
                        out=bk_bc, in_=bk_d[h * C:(h + 1) * C].to_broadcast((P, C)))

                # q, k token-major: (128, 8, 512)
                q_sb = qp.tile([P, TK, C], F32R, tag="q")
                k_sb = kp.tile([P, TK, C], F32R, tag="k")
                for dst, w_sb, bias_bc in (
                    (q_sb, wq_sb, "bq"), (k_sb, wk_sb, "bk"),
                ):
                    for mt in range(TK):
                        ps = ps_mm.tile([P, 512], F32, tag="mm")
                        for kk in range(CK):
                            nc.tensor.matmul(
                                ps,
                                lhsT=(hT[:, kk, mt * P:(mt + 1) * P]),
                                rhs=(w_sb[:, kk, :]),
                                start=(kk == 0), stop=(kk == CK - 1))
                        if bias_bc == "bq" and has_bq:
                            nc.vector.tensor_add(dst[:, mt, :], ps, bq_bc)
                        elif bias_bc == "bk" and has_bk:
                            nc.vector.tensor_add(dst[:, mt, :], ps, bk_bc)
                        else:
                            nc.any.tensor_copy(out=dst[:, mt, :], in_=ps)

                # v channel-major: (128, 4, 1024)
                vT_sb = vp.tile([P, CK, T], F32R, tag="v")
                for ck in range(CK):
                    for tw in range(2):
                        ps = ps_mm.tile([P, 512], F32, tag="mm")
                        for kk in range(CK):
                            nc.tensor.matmul(
                                ps,
                                lhsT=(wv_sb[:, kk, ck * P:(ck + 1) * P]),
                                rhs=(hT[:, kk, tw * 512:(tw + 1) * 512]),
                                start=(kk == 0), stop=(kk == CK - 1))
                        dst = vT_sb[:, ck, tw * 512:(tw + 1) * 512]
                        if has_bv:
                            nc.vector.tensor_scalar_add(
                                dst, ps, bv_col[:, h * CK + ck:h * CK + ck + 1])
                        else:
                            nc.any.tensor_copy(out=dst, in_=ps)

                # logits^T (d on partitions), exp fused with 1/sqrt(T) scale
                et_sb = etp.tile([P, CK, C], F32R, tag="et")
                for dk in range(CK):
                    ps = ps_mm.tile([P, 512], F32, tag="mm")
                    for mt in range(TK):
                        nc.tensor.matmul(
                            ps,
                            lhsT=(k_sb[:, mt, dk * P:(dk + 1) * P]),
                            rhs=(q_sb[:, mt, :]),
                            start=(mt == 0), stop=(mt == TK - 1))
                    nc.scalar.activation(
                        out=et_sb[:, dk, :], in_=ps,
                        func=mybir.ActivationFunctionType.Exp,
                        scale=1.0 / 32.0)

                # softmax denominators, reciprocal per partition (c)
                rs = stat.tile([P, CK], F32, tag="rs")
                for ck in range(CK):
                    ps1 = ps_sm.tile([P, 2], F32, tag="small")
                    for dk in range(CK):
                        nc.tensor.matmul(
                            ps1,
                            lhsT=(et_sb[:, dk, ck * P:(ck + 1) * P]),
                            rhs=(ones2),
                            start=(dk == 0), stop=(dk == CK - 1))
                    nc.vector.reciprocal(rs[:, ck:ck + 1], ps1[:, 0:1])

                # av = (E @ v) / s, channel-major (128, 4, 1024)
                av_sb = avp.tile([P, CK, T], F32R, tag="av")
                for ck in range(CK):
                    for tw in range(2):
                        ps = ps_mm.tile([P, 512], F32, tag="mm")
                        for dk in range(CK):
                            nc.tensor.matmul(
                                ps,
                                lhsT=(et_sb[:, dk, ck * P:(ck + 1) * P]),
                                rhs=(vT_sb[:, dk, tw * 512:(tw + 1) * 512]),
                                start=(dk == 0), stop=(dk == CK - 1))
                        nc.vector.tensor_scalar_mul(
                            av_sb[:, ck, tw * 512:(tw + 1) * 512], ps,
                            rs[:, ck:ck + 1])

                # partial output projection for this head
                for mt in range(TK):
                    ps = ps_mm.tile([P, 512], F32, tag="mm")
                    for ck in range(CK):
                        nc.tensor.matmul(
                            ps,
                            lhsT=(av_sb[:, ck, mt * P:(mt + 1) * P]),
                            rhs=(wo_sb[:, ck, :]),
                            start=(ck == 0), stop=(ck == CK - 1))
                    if h == 0:
                        nc.any.tensor_copy(out=oacc[:, mt, :], in_=ps)
                    elif h < H_RUN - 1:
                        nc.vector.tensor_add(oacc[:, mt, :], oacc[:, mt, :], ps)
                    else:
                        # last head: fold in residual and store immediately
                        ob = obp.tile([P, C], F32, tag="ob")
                        nc.vector.tensor_add(ob, oacc[:, mt, :], ps)
                        nc.vector.tensor_add(ob, ob, x_sb[:, mt, :])
                        if has_bo:
                            nc.vector.tensor_add(ob, ob, bo_bc)
                        nc.sync.dma_start(out=out_view[:, mt, :], in_=ob)

    nc.compile()
    _PROGRAM_CACHE[key] = nc
    return nc


def kernel(x, gn_scale, gn_bias, w_qkv, b_qkv, w_out, b_out, **_unused):
    x = np.ascontiguousarray(np.asarray(x, np.float32)).reshape(B, T, C)
    gn_scale = np.asarray(gn_scale, np.float32)
    gn_bias = np.asarray(gn_bias, np.float32)
    w_qkv = np.asarray(w_qkv, np.float32)
    b_qkv = np.asarray(b_qkv, np.float32)
    w_out = np.ascontiguousarray(np.asarray(w_out, np.float32))
    b_out = np.asarray(b_out, np.float32)

    # de-interleave qkv weights: column j of w_qkv maps to
    # (head = j // (C*3), channel = (j // 3) % C, which = j % 3)
    w4 = w_qkv.reshape(C, H, C, 3)
    w_q = np.ascontiguousarray(w4[..., 0].reshape(C, H * C))
    w_k = np.ascontiguousarray(w4[..., 1].reshape(C, H * C))
    w_v = np.ascontiguousarray(w4[..., 2].reshape(C, H * C))
    b4 = b_qkv.reshape(H, C, 3)
    b_q = np.ascontiguousarray(b4[..., 0].reshape(H * C))
    b_k = np.ascontiguousarray(b4[..., 1].reshape(H * C))
    b_v = np.ascontiguousarray(b4[..., 2].reshape(H * C))

    has_bq = bool(np.any(b_q))
    has_bk = bool(np.any(b_k))
    has_bv = bool(np.any(b_v))
    has_bo = bool(np.any(b_out))

    gs_col = np.ascontiguousarray(gn_scale.reshape(CK, P).T)
    gb_col = np.ascontiguousarray(gn_bias.reshape(CK, P).T)
    m1 = np.kron(np.eye(P // GS, dtype=np.float32),
                 np.ones((GS, GS), np.float32))

    nc = _build_program(has_bq, has_bk, has_bv, has_bo)

    shared = {
        "w_q": w_q, "w_k": w_k, "w_v": w_v, "w_o": w_out,
        "gs_col": gs_col, "gb_col": gb_col, "m1": m1,
    }
    if has_bv:
        shared["bv_col"] = np.ascontiguousarray(b_v.reshape(H * CK, P).T)
    if has_bq:
        shared["b_q"] = b_q
    if has_bk:
        shared["b_k"] = b_k
    if has_bo:
        shared["b_o"] = b_out

    in_maps = [dict(shared, x=np.ascontiguousarray(x[i]))
               for i in range(N_CORES)]

    import os
    trace = bool(int(os.environ.get("BASS_KERNEL_TRACE", "0")))
    kwargs = {}
    if trace:
        tmpdir = os.environ.get("BASS_KERNEL_TRACE_DIR")
        if tmpdir:
            os.makedirs(tmpdir, exist_ok=True)
            kwargs["tmpdir"] = tmpdir
    res = run_bass_kernel_spmd(nc, in_maps, list(range(N_CORES)),
                               trace=trace, **kwargs)
    if trace:
        global LAST_EXEC_TIME_NS
        LAST_EXEC_TIME_NS = res.exec_time_ns
        print(f"HW exec time: {res.exec_time_ns} ns")

    out = np.stack([res.results[i]["out"] for i in range(N_CORES)], axis=0)
    return out.reshape(B, HSP, WSP, C)


LAST_EXEC_TIME_NS = None
